# revision 6
# baseline (speedup 1.0000x reference)
"""Trainium2 Bass kernel for nn_FDDiscriminator (batched RBF-Gram MMD loss).

Math (matches reference):
  x, y: (B=512, T=128, C=16).  The reference builds 2(T-1)=254 time-pair
  slices; those are the 128 distinct time slices with weights w_t = 1 for
  t in {0, T-1} and 2 otherwise.  Per slice t:
    Kxx = exp(-d(x_t, x_t)/2),  Kxy = exp(-d(x_t, y_t)/2)   (512x512)
  with d[m,n] = |a_m|^2 + |b_n|^2 - 2 a_m.b_n, and
  out = mean_t,w[(sum(Kxx)-N)/(N(N-1))] - 2*mean_t,w[mean(Kxy)].

Device strategy (8 cores, 16 time slices each):
  d comes from K=20 bf16 matmuls with augmented operands (fp32 PSUM):
    lhsT rows = [a^T(16); 1; 1; hi|a|^2; lo|a|^2]
    rhs  rows = [-2 b^T(16); hi(|b|^2 - 2 ln c); lo(...); 1; 1]
  where a = bf16(x), norms are computed FROM the bf16 values and split
  hi+lo bf16 (d is then the exact distance matrix of the bf16-rounded
  inputs to ~2^-17, so the Kxx diagonal stays ~0), and c is a per-gram
  constant folded into the exponent: exp(-0.5(d - 2 ln c)) = c*exp(-d/2).

  Kxx is symmetric: per slice we compute only the 6 upper-triangle
  128x128 blocks (folded c = 2w: counted twice) plus the 4 diagonal
  blocks (c = w), packed into one (128,1280) PSUM window; Kxy is the
  full gram in one (128,2048) window with c = w*2(N-1)/N.  One ScalarE
  activation per window computes exp in place with accum_out giving the
  per-partition sum; the weighted combination then needs no per-slice
  coefficients on device.  Host does the tiny final reduction in f64:
    out = (C_xx - 512*sum(w) - C_xy) / (N(N-1)) / 254.
"""

import numpy as np
import ml_dtypes

BF16 = ml_dtypes.bfloat16

B = 512          # batch (gram size N)
T = 128          # time slices after dedup
C = 16           # channels
K = C + 4        # augmented contraction dim
NCORES = 8
SPT = T // NCORES  # slices per core = 16
NBLK = B // 128    # 4 row blocks per gram
UPPER = [(i, j) for i in range(4) for j in range(4) if i < j]  # 6 pairs

_CACHE = {}


def _build_bass():
    import concourse.bass as bass
    import concourse.bacc as bacc
    import concourse.tile as tile
    import concourse.mybir as mybir

    f32 = mybir.dt.float32
    bf16 = mybir.dt.bfloat16
    Exp = mybir.ActivationFunctionType.Exp
    nc = bacc.Bacc(
        "TRN2", target_bir_lowering=False, debug=False, num_devices=NCORES
    )

    L_d = nc.dram_tensor("L", (SPT, K, B), bf16, kind="ExternalInput").ap()
    RU_d = nc.dram_tensor("RU", (SPT, K, B), bf16, kind="ExternalInput").ap()
    RD_d = nc.dram_tensor("RD", (SPT, K, B), bf16, kind="ExternalInput").ap()
    RY_d = nc.dram_tensor("RY", (SPT, K, B), bf16, kind="ExternalInput").ap()
    ACC_d = nc.dram_tensor("ACC", (128, 2 * SPT), f32, kind="ExternalOutput").ap()

    with tile.TileContext(nc) as tc:
        with (
            tc.tile_pool(name="ins", bufs=4) as inpool,
            tc.tile_pool(name="ps", bufs=1, space="PSUM") as pspool,
            tc.tile_pool(name="acc", bufs=1) as accpool,
        ):
            acc_t = accpool.tile([128, 2 * SPT], f32)
            for s in range(SPT):
                l_t = inpool.tile([K, B], bf16, tag="l")
                ru_t = inpool.tile([K, B], bf16, tag="ru")
                rd_t = inpool.tile([K, B], bf16, tag="rd")
                ry_t = inpool.tile([K, B], bf16, tag="ry")
                nc.sync.dma_start(l_t[:], L_d[s])
                nc.sync.dma_start(ru_t[:], RU_d[s])
                nc.sync.dma_start(rd_t[:], RD_d[s])
                nc.sync.dma_start(ry_t[:], RY_d[s])

                # xx: 6 upper-triangle blocks (x2w) + 4 diagonal blocks (xw)
                pxx = pspool.tile([128, 1536], f32, tag="psxx")
                for k, (i, j) in enumerate(UPPER):
                    nc.tensor.matmul(
                        pxx[:, 128 * k : 128 * (k + 1)],
                        lhsT=l_t[:, 128 * i : 128 * (i + 1)],
                        rhs=ru_t[:, 128 * j : 128 * (j + 1)],
                        start=True,
                        stop=True,
                    )
                for i in range(4):
                    nc.tensor.matmul(
                        pxx[:, 768 + 128 * i : 768 + 128 * (i + 1)],
                        lhsT=l_t[:, 128 * i : 128 * (i + 1)],
                        rhs=rd_t[:, 128 * i : 128 * (i + 1)],
                        start=True,
                        stop=True,
                    )
                nc.scalar.activation(
                    pxx[:, 0:1280],
                    pxx[:, 0:1280],
                    Exp,
                    scale=-0.5,
                    accum_out=acc_t[:, 2 * s : 2 * s + 1],
                )

                # xy: full gram (x w*2(N-1)/N)
                pxy = pspool.tile([128, 2048], f32, tag="psxy")
                for i in range(NBLK):
                    nc.tensor.matmul(
                        pxy[:, B * i : B * (i + 1)],
                        lhsT=l_t[:, 128 * i : 128 * (i + 1)],
                        rhs=ry_t[:],
                        start=True,
                        stop=True,
                    )
                nc.scalar.activation(
                    pxy[:],
                    pxy[:],
                    Exp,
                    scale=-0.5,
                    accum_out=acc_t[:, 2 * s + 1 : 2 * s + 2],
                )
            nc.sync.dma_start(ACC_d, acc_t[:])

    nc.compile()
    return nc


def _split_hi_lo(v):
    hi = v.astype(BF16)
    lo = (v - hi.astype(np.float32)).astype(BF16)
    return hi, lo


def _rhs(neg2T, sq_shift):
    """neg2T: (SPT, C, B) bf16; sq_shift: (SPT, B) f32 -> (SPT, K, B) bf16."""
    R = np.empty((SPT, K, B), BF16)
    R[:, :C] = neg2T
    R[:, C], R[:, C + 1] = _split_hi_lo(sq_shift)
    R[:, C + 2] = np.asarray(1.0, BF16)
    R[:, C + 3] = np.asarray(1.0, BF16)
    return R


def _prep_core(xs, ys, w):
    """xs, ys: (B, SPT, C) f32; w: (SPT,) weights -> L, RU, RD, RY bf16."""
    xb = xs.astype(BF16)
    yb = ys.astype(BF16)
    xT = np.ascontiguousarray(xb.transpose(1, 2, 0))  # (SPT, C, B)
    yT = np.ascontiguousarray(yb.transpose(1, 2, 0))
    nxT = (-2.0 * xT.astype(np.float32)).astype(BF16)  # exact 2x scale
    nyT = (-2.0 * yT.astype(np.float32)).astype(BF16)
    sqx = (xb.astype(np.float32) ** 2).sum(axis=2).T  # (SPT, B) f32
    sqy = (yb.astype(np.float32) ** 2).sum(axis=2).T

    L = np.empty((SPT, K, B), BF16)
    L[:, :C] = xT
    L[:, C] = np.asarray(1.0, BF16)
    L[:, C + 1] = np.asarray(1.0, BF16)
    L[:, C + 2], L[:, C + 3] = _split_hi_lo(sqx)

    c_u = 2.0 * w  # upper blocks counted twice
    c_d = w
    c_y = w * (2.0 * (B - 1) / B)
    shift = lambda cs: (2.0 * np.log(cs))[:, None].astype(np.float32)
    RU = _rhs(nxT, sqx - shift(c_u))
    RD = _rhs(nxT, sqx - shift(c_d))
    RY = _rhs(nyT, sqy - shift(c_y))
    return L, RU, RD, RY


def _run(x, y, trace=False, **kw):
    from concourse.bass_utils import run_bass_kernel_spmd

    if "nc" not in _CACHE:
        _CACHE["nc"] = _build_bass()
    nc = _CACHE["nc"]

    w = np.full(T, 2.0)
    w[0] = w[T - 1] = 1.0
    in_maps = []
    for c in range(NCORES):
        sl = slice(c * SPT, (c + 1) * SPT)
        L, RU, RD, RY = _prep_core(x[:, sl, :], y[:, sl, :], w[sl])
        in_maps.append({"L": L, "RU": RU, "RD": RD, "RY": RY})

    return run_bass_kernel_spmd(
        nc, in_maps, list(range(NCORES)), trace=trace, **kw
    )


def _run_with_retries(x, y, trace=False, _trace_kw=None):
    """First execution of a freshly-loaded NEFF occasionally dies with
    NRT_EXEC_UNIT_UNRECOVERABLE; retry, resetting the jax backend in
    between, then fall back to a fresh subprocess."""
    import time as _time

    last = None
    for attempt in range(3):
        try:
            return _run(x, y, trace=trace, **(_trace_kw or {}))
        except Exception as e:  # noqa: BLE001
            last = e
            try:
                import jax

                jax.clear_caches()
                jax.clear_backends()
            except Exception:
                pass
            _time.sleep(2.0)
    # subprocess fallback: fresh process, fresh device session
    import os
    import pickle
    import subprocess
    import sys
    import tempfile

    kdir = os.path.dirname(os.path.abspath(__file__))
    with tempfile.TemporaryDirectory() as td:
        inp = os.path.join(td, "io.pkl")
        with open(inp, "wb") as f:
            pickle.dump({"x": x, "y": y}, f)
        code = (
            "import pickle, sys; sys.path.insert(0, %r); import kernel as km; "
            "d = pickle.load(open(%r, 'rb')); "
            "r = km.kernel(d['x'], d['y']); "
            "pickle.dump(r, open(%r, 'wb'))"
            % (kdir, inp, inp + ".out")
        )
        for attempt in range(2):
            p = subprocess.run(
                [sys.executable, "-c", code], capture_output=True, timeout=1800
            )
            if p.returncode == 0 and os.path.exists(inp + ".out"):
                with open(inp + ".out", "rb") as f:
                    return pickle.load(f)
    raise last


def kernel(x, y, _trace=False, _trace_kw=None):
    x = np.asarray(x, np.float32)
    y = np.asarray(y, np.float32)
    res = _run_with_retries(x, y, trace=_trace, _trace_kw=_trace_kw)
    if isinstance(res, np.floating | np.ndarray):
        return res  # came from the subprocess fallback, already reduced

    c_xx = 0.0
    c_xy = 0.0
    for c in range(NCORES):
        acc = np.asarray(res.results[c]["ACC"], np.float64)  # (128, 2*SPT)
        sums = acc.sum(axis=0)
        c_xx += sums[0::2].sum()
        c_xy += sums[1::2].sum()
    out = (c_xx - 512.0 * 254.0 - c_xy) / (B * (B - 1)) / 254.0
    if _trace:
        kernel.last_results = res
    return np.float32(out)


# revision 8
# speedup vs baseline: 1.0196x; 1.0196x over previous
"""Trainium2 Bass kernel for nn_FDDiscriminator (batched RBF-Gram MMD loss).

Math (matches reference):
  x, y: (B=512, T=128, C=16).  The reference builds 2(T-1)=254 time-pair
  slices; those are the 128 distinct time slices with weights w_t = 1 for
  t in {0, T-1} and 2 otherwise.  Per slice t:
    Kxx = exp(-d(x_t, x_t)/2),  Kxy = exp(-d(x_t, y_t)/2)   (512x512)
  with d[m,n] = |a_m|^2 + |b_n|^2 - 2 a_m.b_n, and
  out = mean_t,w[(sum(Kxx)-N)/(N(N-1))] - 2*mean_t,w[mean(Kxy)].

Device strategy (8 cores, 16 time slices each):
  d comes from K=20 bf16 matmuls with augmented operands (fp32 PSUM):
    lhsT rows = [a^T(16); 1; 1; hi|a|^2; lo|a|^2]
    rhs  rows = [-2 b^T(16); hi(|b|^2 - 2 ln c); lo(...); 1; 1]
  where a = bf16(x), norms are computed FROM the bf16 values and split
  hi+lo bf16 (d is then the exact distance matrix of the bf16-rounded
  inputs to ~2^-17, so the Kxx diagonal stays ~0), and c is a per-gram
  constant folded into the exponent: exp(-0.5(d - 2 ln c)) = c*exp(-d/2).

  Kxx is symmetric: per slice we compute only the 6 upper-triangle
  128x128 blocks (folded c = 2w: counted twice) plus the 4 diagonal
  blocks (c = w), packed into one (128,1280) PSUM window; Kxy is the
  full gram in one (128,2048) window with c = w*2(N-1)/N.  One ScalarE
  activation per window computes exp in place with accum_out giving the
  per-partition sum; the weighted combination then needs no per-slice
  coefficients on device.  Host does the tiny final reduction in f64:
    out = (C_xx - 512*sum(w) - C_xy) / (N(N-1)) / 254.
"""

import numpy as np
import ml_dtypes

BF16 = ml_dtypes.bfloat16

B = 512          # batch (gram size N)
T = 128          # time slices after dedup
C = 16           # channels
K = C + 4        # augmented contraction dim
NCORES = 8
SPT = T // NCORES  # slices per core = 16
NBLK = B // 128    # 4 row blocks per gram
UPPER = [(i, j) for i in range(4) for j in range(4) if i < j]  # 6 pairs

_CACHE = {}


def _build_bass():
    import concourse.bass as bass
    import concourse.bacc as bacc
    import concourse.tile as tile
    import concourse.mybir as mybir

    f32 = mybir.dt.float32
    bf16 = mybir.dt.bfloat16
    Exp = mybir.ActivationFunctionType.Exp
    nc = bacc.Bacc(
        "TRN2", target_bir_lowering=False, debug=False, num_devices=NCORES
    )

    L_d = nc.dram_tensor("L", (SPT, K, B), bf16, kind="ExternalInput").ap()
    RU_d = nc.dram_tensor("RU", (SPT, K, B), bf16, kind="ExternalInput").ap()
    RD_d = nc.dram_tensor("RD", (SPT, K, B), bf16, kind="ExternalInput").ap()
    RY_d = nc.dram_tensor("RY", (SPT, K, B), bf16, kind="ExternalInput").ap()
    ACC_d = nc.dram_tensor("ACC", (128, 2 * SPT), f32, kind="ExternalOutput").ap()

    with tile.TileContext(nc) as tc:
        with (
            tc.tile_pool(name="ins", bufs=4) as inpool,
            tc.tile_pool(name="ps", bufs=1, space="PSUM") as pspool,
            tc.tile_pool(name="acc", bufs=1) as accpool,
        ):
            acc_t = accpool.tile([128, 2 * SPT], f32)
            for s in range(SPT):
                l_t = inpool.tile([K, B], bf16, tag="l")
                ry_t = inpool.tile([K, B], bf16, tag="ry")
                ru_t = inpool.tile([K, B], bf16, tag="ru")
                rd_t = inpool.tile([K, B], bf16, tag="rd")
                nc.sync.dma_start(l_t[:], L_d[s])
                nc.sync.dma_start(ry_t[:], RY_d[s])
                nc.sync.dma_start(ru_t[:], RU_d[s])
                nc.sync.dma_start(rd_t[:], RD_d[s])

                # xy first: needs only the first two DMAs, so the pipeline
                # ramps one DMA earlier.  Full gram (x w*2(N-1)/N).
                pxy = pspool.tile([128, 2048], f32, tag="psxy")
                for i in range(NBLK):
                    nc.tensor.matmul(
                        pxy[:, B * i : B * (i + 1)],
                        lhsT=l_t[:, 128 * i : 128 * (i + 1)],
                        rhs=ry_t[:],
                        start=True,
                        stop=True,
                    )
                nc.scalar.activation(
                    pxy[:],
                    pxy[:],
                    Exp,
                    scale=-0.5,
                    accum_out=acc_t[:, 2 * s : 2 * s + 1],
                )

                # xx window, (128, 1152):
                #   [0,768):    6 upper-triangle 128-blocks      (x 2w)
                #   [768,896):  4 Q quarters (64x64, rows [0:64) x cols
                #               [64:128) of each diag block), stacked two
                #               per 64-col range at partitions 0/64 (x 2w)
                #   [896,1152): 8 diag 64-sub-blocks, stacked two per
                #               64-col range at partitions 0/64   (x w)
                pxx = pspool.tile([128, 1536], f32, tag="psxx")
                for k, (i, j) in enumerate(UPPER):
                    nc.tensor.matmul(
                        pxx[:, 128 * k : 128 * (k + 1)],
                        lhsT=l_t[:, 128 * i : 128 * (i + 1)],
                        rhs=ru_t[:, 128 * j : 128 * (j + 1)],
                        start=True,
                        stop=True,
                    )
                for i in range(4):
                    half = 64 * (i % 2)
                    col = 768 + 64 * (i // 2)
                    nc.tensor.matmul(
                        pxx[half : half + 64, col : col + 64],
                        lhsT=l_t[:, 128 * i : 128 * i + 64],
                        rhs=ru_t[:, 128 * i + 64 : 128 * (i + 1)],
                        start=True,
                        stop=True,
                    )
                for i in range(4):
                    for h in range(2):
                        nc.tensor.matmul(
                            pxx[64 * h : 64 * (h + 1), 896 + 64 * i : 960 + 64 * i],
                            lhsT=l_t[:, 128 * i + 64 * h : 128 * i + 64 * (h + 1)],
                            rhs=rd_t[:, 128 * i + 64 * h : 128 * i + 64 * (h + 1)],
                            start=True,
                            stop=True,
                        )
                nc.scalar.activation(
                    pxx[:, 0:1152],
                    pxx[:, 0:1152],
                    Exp,
                    scale=-0.5,
                    accum_out=acc_t[:, 2 * s + 1 : 2 * s + 2],
                )
            nc.sync.dma_start(ACC_d, acc_t[:])

    nc.compile()
    return nc


def _split_hi_lo(v):
    hi = v.astype(BF16)
    lo = (v - hi.astype(np.float32)).astype(BF16)
    return hi, lo


def _rhs(neg2T, sq_shift):
    """neg2T: (SPT, C, B) bf16; sq_shift: (SPT, B) f32 -> (SPT, K, B) bf16."""
    R = np.empty((SPT, K, B), BF16)
    R[:, :C] = neg2T
    R[:, C], R[:, C + 1] = _split_hi_lo(sq_shift)
    R[:, C + 2] = np.asarray(1.0, BF16)
    R[:, C + 3] = np.asarray(1.0, BF16)
    return R


def _prep_core(xs, ys, w):
    """xs, ys: (B, SPT, C) f32; w: (SPT,) weights -> L, RU, RD, RY bf16."""
    xb = xs.astype(BF16)
    yb = ys.astype(BF16)
    xT = np.ascontiguousarray(xb.transpose(1, 2, 0))  # (SPT, C, B)
    yT = np.ascontiguousarray(yb.transpose(1, 2, 0))
    nxT = (-2.0 * xT.astype(np.float32)).astype(BF16)  # exact 2x scale
    nyT = (-2.0 * yT.astype(np.float32)).astype(BF16)
    sqx = (xb.astype(np.float32) ** 2).sum(axis=2).T  # (SPT, B) f32
    sqy = (yb.astype(np.float32) ** 2).sum(axis=2).T

    L = np.empty((SPT, K, B), BF16)
    L[:, :C] = xT
    L[:, C] = np.asarray(1.0, BF16)
    L[:, C + 1] = np.asarray(1.0, BF16)
    L[:, C + 2], L[:, C + 3] = _split_hi_lo(sqx)

    c_u = 2.0 * w  # upper blocks counted twice
    c_d = w
    c_y = w * (2.0 * (B - 1) / B)
    shift = lambda cs: (2.0 * np.log(cs))[:, None].astype(np.float32)
    RU = _rhs(nxT, sqx - shift(c_u))
    RD = _rhs(nxT, sqx - shift(c_d))
    RY = _rhs(nyT, sqy - shift(c_y))
    return L, RU, RD, RY


def _run(x, y, trace=False, **kw):
    from concourse.bass_utils import run_bass_kernel_spmd

    if "nc" not in _CACHE:
        _CACHE["nc"] = _build_bass()
    nc = _CACHE["nc"]

    w = np.full(T, 2.0)
    w[0] = w[T - 1] = 1.0
    in_maps = []
    for c in range(NCORES):
        sl = slice(c * SPT, (c + 1) * SPT)
        L, RU, RD, RY = _prep_core(x[:, sl, :], y[:, sl, :], w[sl])
        in_maps.append({"L": L, "RU": RU, "RD": RD, "RY": RY})

    return run_bass_kernel_spmd(
        nc, in_maps, list(range(NCORES)), trace=trace, **kw
    )


def _run_with_retries(x, y, trace=False, _trace_kw=None):
    """First execution of a freshly-loaded NEFF occasionally dies with
    NRT_EXEC_UNIT_UNRECOVERABLE; retry, resetting the jax backend in
    between, then fall back to a fresh subprocess."""
    import time as _time

    last = None
    for attempt in range(3):
        try:
            return _run(x, y, trace=trace, **(_trace_kw or {}))
        except Exception as e:  # noqa: BLE001
            last = e
            try:
                import jax

                jax.clear_caches()
                jax.clear_backends()
            except Exception:
                pass
            _time.sleep(2.0)
    # subprocess fallback: fresh process, fresh device session
    import os
    import pickle
    import subprocess
    import sys
    import tempfile

    kdir = os.path.dirname(os.path.abspath(__file__))
    with tempfile.TemporaryDirectory() as td:
        inp = os.path.join(td, "io.pkl")
        with open(inp, "wb") as f:
            pickle.dump({"x": x, "y": y}, f)
        code = (
            "import pickle, sys; sys.path.insert(0, %r); import kernel as km; "
            "d = pickle.load(open(%r, 'rb')); "
            "r = km.kernel(d['x'], d['y']); "
            "pickle.dump(r, open(%r, 'wb'))"
            % (kdir, inp, inp + ".out")
        )
        for attempt in range(2):
            p = subprocess.run(
                [sys.executable, "-c", code], capture_output=True, timeout=1800
            )
            if p.returncode == 0 and os.path.exists(inp + ".out"):
                with open(inp + ".out", "rb") as f:
                    return pickle.load(f)
    raise last


def kernel(x, y, _trace=False, _trace_kw=None):
    x = np.asarray(x, np.float32)
    y = np.asarray(y, np.float32)
    res = _run_with_retries(x, y, trace=_trace, _trace_kw=_trace_kw)
    if isinstance(res, np.floating | np.ndarray):
        return res  # came from the subprocess fallback, already reduced

    c_xx = 0.0
    c_xy = 0.0
    for c in range(NCORES):
        acc = np.asarray(res.results[c]["ACC"], np.float64)  # (128, 2*SPT)
        sums = acc.sum(axis=0)
        c_xy += sums[0::2].sum()  # xy windows run first per slice
        c_xx += sums[1::2].sum()
    out = (c_xx - 512.0 * 254.0 - c_xy) / (B * (B - 1)) / 254.0
    if _trace:
        kernel.last_results = res
    return np.float32(out)
